# revision 8
# baseline (speedup 1.0000x reference)
"""DecompGridv3 embedding lookup on 8 Trainium2 NeuronCores.

Strategy (data-parallel over B=1M query points, 128K/core):
The kernel is bound by the SWDGE (Pool/Q7) per-gather-element cost
(~7-10ns per gathered element, regardless of element bytes), so the
design minimizes gather ELEMENTS per point (5):
  * grid:  fp16, zy-shingled rows of 128 fp16 (256B): row(z,y,x) =
    [v(z,y,x), v(z,y+1,x), v(z+1,y,x), v(z+1,y+1,x)].  One 512B
    overlap-read (rows x0, x0+1) fetches all 8 trilinear corners.
  * planes: fp16, xy-shingled bricks of 128 fp16 (256B): brick(y,x) =
    [P(y,x), P(y,x+1), P(y+1,x), P(y+1,x+1)] - the full 2x2 patch in
    ONE 256B read per plane.
  * line:  f32 pair-shingled rows of 64 f32 (256B), batched dma_gather.
Compute on DVE in fp16 (2x tensor_tensor mode): corner-weight outer
products, flat multiply + pairwise tree adds (tensor_reduce is 1x-only
so trees win), final product chain in f32 to dodge fp16 subnormal
flushing near the output scale.
"""

import numpy as np

import concourse.bacc as bacc
import concourse.tile as tile
import concourse.mybir as mybir
from concourse.bass import AP, IndirectOffsetOnAxis
from concourse.bass_utils import run_bass_kernel_spmd

F32 = mybir.dt.float32
F16 = mybir.dt.float16
I32 = mybir.dt.int32
I16 = mybir.dt.int16
ALU = mybir.AluOpType

NF = 32          # features
D3 = 128         # 3D grid res
P2 = 384         # plane res
L1 = 256         # line length
B = 1 << 20      # total points
NCORES = 8
BCORE = B // NCORES          # 131072 points per core
JTOT = BCORE // 128          # 1024 free-dim point columns per core
CHUNK_J = 16                 # j-columns per chunk
NCHUNK = JTOT // CHUNK_J     # 64
TILE_S = 4                   # j-columns per compute tile
NTILE = CHUNK_J // TILE_S    # 4

# fp16 table layout (rows of 128 fp16 = 256B)
GRID_ROWS = D3 * D3 * D3                # zy-shingled rows
PLANE_ROWS = P2 * P2                    # xy-shingled bricks
BASE_G = 0
BASE_P01 = GRID_ROWS
BASE_P02 = BASE_P01 + PLANE_ROWS
BASE_P12 = BASE_P02 + PLANE_ROWS
TAB_ROWS = BASE_P12 + PLANE_ROWS + 1    # +1 row slack for overlap reads


def build_bass():
    nc = bacc.Bacc("TRN2", target_bir_lowering=False, debug=False,
                   num_devices=NCORES)
    xin = nc.dram_tensor("xin", [128, JTOT * 4], F32, kind="ExternalInput")
    tab = nc.dram_tensor("tab", [TAB_ROWS, 128], F16, kind="ExternalInput")
    ltab = nc.dram_tensor("ltab", [L1, 2 * NF], F32, kind="ExternalInput")
    xl16 = nc.dram_tensor("xl16", [16, JTOT * 8], F32, kind="ExternalInput")
    out = nc.dram_tensor("out", [128, JTOT * NF], F32, kind="ExternalOutput")

    J = CHUNK_J
    with tile.TileContext(nc) as tc:
        import contextlib
        with contextlib.ExitStack() as ctx:
            xp = ctx.enter_context(tc.tile_pool(name="xp", bufs=2))
            wp = ctx.enter_context(tc.tile_pool(name="wp", bufs=2))
            sp = ctx.enter_context(tc.tile_pool(name="sp", bufs=2))
            op = ctx.enter_context(tc.tile_pool(name="op", bufs=3))
            gp = ctx.enter_context(tc.tile_pool(name="gp", bufs=3))
            mp = ctx.enter_context(tc.tile_pool(name="mp", bufs=2))
            rp = ctx.enter_context(tc.tile_pool(name="rp", bufs=2))
            lp = ctx.enter_context(tc.tile_pool(name="lp", bufs=3))
            lip = ctx.enter_context(tc.tile_pool(name="lip", bufs=2))

            for c in range(NCHUNK):
                j0 = c * J
                # ---- load x chunk: [128, J, 4]
                xs = xp.tile([128, J, 4], F32, tag="xs")
                nc.sync.dma_start(
                    xs[:], xin.ap()[:, j0 * 4:(j0 + J) * 4]
                           .rearrange("p (j c) -> p j c", c=4))

                def coord(k):
                    return xs[:, :, k:k + 1].rearrange("p j o -> p (j o)")

                # ---- per-coord floors and fracs
                def floorfrac(fv, tg):
                    ri = sp.tile([128, J], I32, tag="ffi", name="ri")
                    nc.vector.tensor_copy(ri[:], fv[:])          # round
                    rf = sp.tile([128, J], F32, tag="ffr", name="rf")
                    nc.vector.tensor_copy(rf[:], ri[:])
                    m = sp.tile([128, J], F32, tag="ffm", name="m")
                    nc.vector.tensor_tensor(out=m[:], in0=rf[:], in1=fv[:],
                                            op=ALU.is_gt)
                    fl = sp.tile([128, J], F32, tag=tg + "l", name="fl")
                    nc.vector.tensor_sub(fl[:], rf[:], m[:])
                    w = sp.tile([128, J], F32, tag=tg + "w", name="w")
                    nc.vector.tensor_sub(w[:], fv[:], fl[:])
                    return fl, w

                fl3, w3, fl2, w2 = [], [], [], []
                for k in range(3):
                    t = sp.tile([128, J], F32, tag="t")
                    nc.vector.tensor_scalar(out=t[:], in0=coord(k),
                                            scalar1=1.0, scalar2=0.5,
                                            op0=ALU.add, op1=ALU.mult)
                    f3 = sp.tile([128, J], F32, tag="f3")
                    nc.vector.tensor_scalar(out=f3[:], in0=t[:],
                                            scalar1=float(D3 - 1), scalar2=None,
                                            op0=ALU.mult)
                    f2 = sp.tile([128, J], F32, tag="f2")
                    nc.vector.tensor_scalar(out=f2[:], in0=t[:],
                                            scalar1=float(P2 - 1), scalar2=None,
                                            op0=ALU.mult)
                    a, b_ = floorfrac(f3, f"f3{k}")
                    fl3.append(a); w3.append(b_)
                    a, b_ = floorfrac(f2, f"f2{k}")
                    fl2.append(a); w2.append(b_)

                # ---- gather offsets (fp32 -> int32), in 256B-row units
                # grid row index = z*D3*D3 + y*D3 + x
                offg = op.tile([128, J], I32, tag="offg")
                b_ = sp.tile([128, J], F32, tag="gb")
                nc.vector.tensor_scalar(out=b_[:], in0=fl3[1],
                                        scalar1=float(D3), scalar2=None,
                                        op0=ALU.mult)
                a_ = sp.tile([128, J], F32, tag="ga")
                nc.vector.scalar_tensor_tensor(
                    out=a_[:], in0=fl3[2], scalar=float(D3 * D3), in1=b_[:],
                    op0=ALU.mult, op1=ALU.add)
                g00 = sp.tile([128, J], F32, tag="g00")
                nc.vector.tensor_add(g00[:], a_[:], fl3[0])
                nc.vector.tensor_copy(offg[:], g00[:])

                offp = op.tile([128, 3 * J], I32, tag="offp")
                for p_i, (ky, kx, base) in enumerate(
                        ((1, 0, BASE_P01), (2, 0, BASE_P02), (2, 1, BASE_P12))):
                    r_ = sp.tile([128, J], F32, tag="pr")
                    nc.vector.scalar_tensor_tensor(
                        out=r_[:], in0=fl2[ky], scalar=float(P2), in1=fl2[kx],
                        op0=ALU.mult, op1=ALU.add)
                    r2 = sp.tile([128, J], F32, tag="pr2")
                    nc.vector.tensor_scalar(out=r2[:], in0=r_[:],
                                            scalar1=1.0, scalar2=float(base),
                                            op0=ALU.mult, op1=ALU.add)
                    nc.vector.tensor_copy(offp[:, p_i * J:(p_i + 1) * J], r2[:])

                # ---- weight pairs (fp16) and corner products
                def mkpair(w, tag):
                    pr = wp.tile([128, J, 2], F16, tag=tag)
                    nc.vector.tensor_copy(pr[:, :, 1:2],
                                          w[:].to_broadcast([128, J, 1]))
                    nc.vector.tensor_scalar(
                        out=pr[:, :, 0:1],
                        in0=w[:].to_broadcast([128, J, 1]),
                        scalar1=1.0, scalar2=-1.0,
                        op0=ALU.subtract, op1=ALU.mult)
                    return pr

                wzp = mkpair(w3[2], "wzp")
                wyp = mkpair(w3[1], "wyp")
                wxp = mkpair(w3[0], "wxp")
                w2xp = mkpair(w2[0], "w2xp")
                w2yp = mkpair(w2[1], "w2yp")
                w2zp = mkpair(w2[2], "w2zp")

                def outer2(pa, pb, tag):
                    # out[p, j, a, b] = pa[p,j,a] * pb[p,j,b]
                    o = wp.tile([128, J, 2, 2], F16, tag=tag)
                    nc.vector.tensor_mul(
                        o[:], pa[:].to_broadcast([128, J, 2, 2]),
                        AP(pb[:].tensor, pb[:].offset,
                           [list(pb[:].ap[0]), list(pb[:].ap[1]),
                            [0, 2], list(pb[:].ap[2])]))
                    return o

                # grid corner order within 512B read: (dx, dz, dy)
                wzy = outer2(wzp, wyp, "wzy")          # [p,J,dz,dy]
                w8 = wp.tile([128, J, 2, 2, 2], F16, tag="w8")  # [dx,dz,dy]
                nc.vector.tensor_mul(
                    w8[:],
                    AP(wxp[:].tensor, wxp[:].offset,
                       [list(wxp[:].ap[0]), list(wxp[:].ap[1]),
                        list(wxp[:].ap[2]), [0, 2], [0, 2]]),
                    AP(wzy[:].tensor, wzy[:].offset,
                       [list(wzy[:].ap[0]), list(wzy[:].ap[1]), [0, 2],
                        list(wzy[:].ap[2]), list(wzy[:].ap[3])]))
                # plane brick corner order: (dy, dx) [row-pairs: y, y+1]
                # brick = [P(y,x), P(y,x+1), P(y+1,x), P(y+1,x+1)]
                w401 = outer2(w2yp, w2xp, "w401")
                w402 = outer2(w2zp, w2xp, "w402")
                w412 = outer2(w2zp, w2yp, "w412")

                # line weights stay f32
                flv = sp.tile([128, J], F32, tag="flv")
                nc.vector.tensor_scalar(out=flv[:], in0=coord(3),
                                        scalar1=float(L1), scalar2=None,
                                        op0=ALU.mult)
                _, wl = floorfrac(flv, "flx")
                wlp = wp.tile([128, J, 2], F32, tag="wlp")
                nc.vector.tensor_copy(wlp[:, :, 1:2],
                                      wl[:].to_broadcast([128, J, 1]))
                nc.vector.tensor_scalar(
                    out=wlp[:, :, 0:1],
                    in0=wl[:].to_broadcast([128, J, 1]),
                    scalar1=1.0, scalar2=-1.0,
                    op0=ALU.subtract, op1=ALU.mult)

                # ---- line: batched dma_gather (int16 idx on 16 partitions)
                li_f = lip.tile([16, J * 8], F32, tag="lif")
                nc.sync.dma_start(
                    li_f[:], xl16.ap()[:, c * J * 8:(c + 1) * J * 8])
                lfv = lip.tile([16, J * 8], F32, tag="lfv")
                nc.vector.tensor_scalar(out=lfv[:], in0=li_f[:],
                                        scalar1=float(L1), scalar2=None,
                                        op0=ALU.mult)
                lri = lip.tile([16, J * 8], I32, tag="lri")
                nc.vector.tensor_copy(lri[:], lfv[:])
                lrf = lip.tile([16, J * 8], F32, tag="lrf")
                nc.vector.tensor_copy(lrf[:], lri[:])
                lm = lip.tile([16, J * 8], F32, tag="lm")
                nc.vector.tensor_tensor(out=lm[:], in0=lrf[:], in1=lfv[:],
                                        op=ALU.is_gt)
                lfl = lip.tile([16, J * 8], F32, tag="lfl")
                nc.vector.tensor_sub(lfl[:], lrf[:], lm[:])
                lidx = lip.tile([128, J * 8], I16, tag="lidx")
                nc.vector.tensor_copy(lidx[0:16, :], lfl[:])
                for d in range(3):
                    n = 16 << d
                    nc.sync.dma_start(lidx[n:2 * n, :], lidx[0:n, :])
                ld = lp.tile([128, J, 2 * NF], F32, tag="ld")
                nc.gpsimd.dma_gather(
                    out_ap=ld[:], in_ap=ltab.ap(), idxs_ap=lidx[:],
                    num_idxs=J * 128, num_idxs_reg=J * 128,
                    elem_size=2 * NF, single_packet=False)

                # ---- per-j-column indirect gathers (fp16)
                g3c = gp.tile([128, J, 256], F16, tag="g3")
                gplc = [gp.tile([128, J, 128], F16, tag=f"gp{i}",
                                name=f"gp{i}") for i in range(3)]
                for j in range(J):
                    nc.gpsimd.indirect_dma_start(
                        out=g3c[:, j:j + 1, :].rearrange("p a b -> p (a b)"),
                        out_offset=None, in_=tab.ap(),
                        in_offset=IndirectOffsetOnAxis(
                            ap=offg[:, j:j + 1], axis=0))
                    for p_i in range(3):
                        nc.gpsimd.indirect_dma_start(
                            out=gplc[p_i][:, j:j + 1, :]
                                .rearrange("p a b -> p (a b)"),
                            out_offset=None, in_=tab.ap(),
                            in_offset=IndirectOffsetOnAxis(
                                ap=offp[:, p_i * J + j:p_i * J + j + 1],
                                axis=0))

                # ---- compute per tile (fp16 mult + tree adds)
                oc = rp.tile([128, J, NF], F32, tag="oc")
                for s in range(NTILE):
                    u0 = s * TILE_S
                    S = TILE_S

                    def wsum16(gt, wt, ncorn, tag):
                        # gt: [p,S,ncorn*NF] f16; wt [p,J,ncorn] f16
                        m = mp.tile([128, S, ncorn, NF], F16, tag=tag)
                        nc.vector.tensor_mul(
                            m[:],
                            gt.rearrange("p u (c f) -> p u c f",
                                         c=ncorn, f=NF),
                            AP(wt.tensor, wt.offset + u0 * ncorn,
                               [list(wt.ap[0]), [ncorn, S],
                                [1, ncorn], [0, NF]]))
                        # tree-add over corners
                        cur = m
                        n = ncorn
                        while n > 1:
                            h = mp.tile([128, S, n // 2, NF], F16,
                                        tag=tag + f"h{n}")
                            nc.vector.tensor_add(
                                h[:], cur[:, :, 0:n // 2, :],
                                cur[:, :, n // 2:n, :])
                            cur = h
                            n //= 2
                        return cur[:, :, 0:1, :].rearrange(
                            "p u o f -> p u (o f)")

                    f3r = wsum16(g3c[:, u0:u0 + S, :],
                                 w8[:].rearrange("p j a b c -> p j (a b c)"),
                                 8, "m8")
                    p01r = wsum16(gplc[0][:, u0:u0 + S, :],
                                  w401[:].rearrange("p j a b -> p j (a b)"),
                                  4, "m01")
                    p02r = wsum16(gplc[1][:, u0:u0 + S, :],
                                  w402[:].rearrange("p j a b -> p j (a b)"),
                                  4, "m02")
                    p12r = wsum16(gplc[2][:, u0:u0 + S, :],
                                  w412[:].rearrange("p j a b -> p j (a b)"),
                                  4, "m12")

                    # line weighted sum in f32
                    ml = mp.tile([128, S, 2, NF], F32, tag="ml")
                    nc.vector.tensor_mul(
                        ml[:],
                        ld[:, u0:u0 + S, :].rearrange(
                            "p u (c f) -> p u c f", c=2, f=NF),
                        AP(wlp[:].tensor, wlp[:].offset + u0 * 2,
                           [list(wlp[:].ap[0]), [2, S], [1, 2], [0, NF]]))
                    flr = mp.tile([128, S, NF], F32, tag="mlr")
                    nc.vector.tensor_add(flr[:], ml[:, :, 0:1, :].rearrange(
                        "p u o f -> p u (o f)"), ml[:, :, 1:2, :].rearrange(
                        "p u o f -> p u (o f)"))

                    o1 = rp.tile([128, S, NF], F16, tag="o1")
                    nc.vector.tensor_mul(o1[:], f3r, p01r)
                    o2 = rp.tile([128, S, NF], F16, tag="o2")
                    nc.vector.tensor_mul(o2[:], p02r, p12r)
                    o3 = rp.tile([128, S, NF], F32, tag="o3")
                    nc.vector.tensor_mul(o3[:], o1[:], o2[:])
                    nc.vector.tensor_mul(oc[:, u0:u0 + S, :], o3[:], flr[:])
                nc.sync.dma_start(
                    out.ap()[:, j0 * NF:(j0 + J) * NF],
                    oc[:].rearrange("p u f -> p (u f)"))

    nc.compile()
    return nc


def _prep_tables(grid3d, plane01, plane02, plane12, line0):
    # grid: zy-shingled fp16 rows: row(z,y,x) = [v(z,y,x), v(z,y+1,x),
    #   v(z+1,y,x), v(z+1,y+1,x)], each 32 fp16
    g = np.ascontiguousarray(grid3d.transpose(1, 2, 3, 0)).astype(np.float16)
    S = np.zeros((D3, D3, D3, 4, NF), np.float16)
    S[:, :, :, 0] = g
    S[:, :-1, :, 1] = g[:, 1:]
    S[:-1, :, :, 2] = g[1:]
    S[:-1, :-1, :, 3] = g[1:, 1:]
    grid_t = S.reshape(GRID_ROWS, 128)
    del S

    def shingle_plane(p):
        pt = np.ascontiguousarray(p.transpose(1, 2, 0)).astype(np.float16)
        ps = np.zeros((P2, P2, 4, NF), np.float16)
        ps[:, :, 0] = pt
        ps[:, :-1, 1] = pt[:, 1:]
        ps[:-1, :, 2] = pt[1:]
        ps[:-1, :-1, 3] = pt[1:, 1:]
        return ps.reshape(PLANE_ROWS, 128)

    tab = np.concatenate([grid_t, shingle_plane(plane01),
                          shingle_plane(plane02), shingle_plane(plane12),
                          np.zeros((1, 128), np.float16)], axis=0)
    lt = np.ascontiguousarray(line0.T)                        # (256, 32)
    ls = np.empty((L1, 2 * NF), np.float32)
    ls[:, :NF] = lt
    ls[:-1, NF:] = lt[1:]
    ls[-1, NF:] = lt[-1]
    return tab, ls


_NC_CACHE = {}


def kernel(x, grid3d, plane01, plane02, plane12, line0):
    x = np.asarray(x, np.float32)
    tab, ls = _prep_tables(np.asarray(grid3d, np.float32),
                           np.asarray(plane01, np.float32),
                           np.asarray(plane02, np.float32),
                           np.asarray(plane12, np.float32),
                           np.asarray(line0, np.float32))
    if "nc" not in _NC_CACHE:
        _NC_CACHE["nc"] = build_bass()
    nc = _NC_CACHE["nc"]

    in_maps = []
    for cix in range(NCORES):
        xc = x[cix * BCORE:(cix + 1) * BCORE]          # (131072, 4)
        xin = np.ascontiguousarray(xc.reshape(128, JTOT * 4))
        x3 = xc[:, 3].reshape(128, JTOT)               # [p, jglob]
        x3g = x3.reshape(8, 16, NCHUNK, CHUNK_J)       # [k, q, c, j]
        xl16 = np.ascontiguousarray(
            x3g.transpose(1, 2, 3, 0).reshape(16, JTOT * 8))
        in_maps.append({"xin": xin, "tab": tab, "ltab": ls, "xl16": xl16})

    res = run_bass_kernel_spmd(nc, in_maps, core_ids=list(range(NCORES)))
    outs = [r["out"].reshape(BCORE, NF) for r in res.results]
    return np.concatenate(outs, axis=0)


# revision 9
# speedup vs baseline: 1.1030x; 1.1030x over previous
"""DecompGridv3 embedding lookup on 8 Trainium2 NeuronCores.

Strategy (data-parallel over B=1M query points, 128K/core):
The kernel is bound by the SWDGE (Pool/Q7) per-gather-element cost
(~7-10ns per gathered element, regardless of element bytes), so the
design minimizes gather ELEMENTS per point (5):
  * grid:  fp16, zy-shingled rows of 128 fp16 (256B): row(z,y,x) =
    [v(z,y,x), v(z,y+1,x), v(z+1,y,x), v(z+1,y+1,x)].  One 512B
    overlap-read (rows x0, x0+1) fetches all 8 trilinear corners.
  * planes: fp16, xy-shingled bricks of 128 fp16 (256B): brick(y,x) =
    [P(y,x), P(y,x+1), P(y+1,x), P(y+1,x+1)] - the full 2x2 patch in
    ONE 256B read per plane.
  * line:  f32 pair-shingled rows of 64 f32 (256B), batched dma_gather.
Compute on DVE in fp16 (2x tensor_tensor mode): corner-weight outer
products, flat multiply + pairwise tree adds (tensor_reduce is 1x-only
so trees win), final product chain in f32 to dodge fp16 subnormal
flushing near the output scale.
"""

import numpy as np

import concourse.bacc as bacc
import concourse.tile as tile
import concourse.mybir as mybir
from concourse.bass import AP, IndirectOffsetOnAxis
from concourse.bass_utils import run_bass_kernel_spmd

F32 = mybir.dt.float32
F16 = mybir.dt.float16
I32 = mybir.dt.int32
I16 = mybir.dt.int16
ALU = mybir.AluOpType

NF = 32          # features
D3 = 128         # 3D grid res
P2 = 384         # plane res
L1 = 256         # line length
B = 1 << 20      # total points
NCORES = 8
BCORE = B // NCORES          # 131072 points per core
JTOT = BCORE // 128          # 1024 free-dim point columns per core
CHUNK_J = 16                 # j-columns per chunk
NCHUNK = JTOT // CHUNK_J     # 64
TILE_S = 8                   # j-columns per compute tile
NTILE = CHUNK_J // TILE_S    # 4

# fp16 table layout (rows of 128 fp16 = 256B)
GRID_ROWS = D3 * D3 * D3                # zy-shingled rows
PLANE_ROWS = P2 * P2                    # xy-shingled bricks
BASE_G = 0
BASE_P01 = GRID_ROWS
BASE_P02 = BASE_P01 + PLANE_ROWS
BASE_P12 = BASE_P02 + PLANE_ROWS
TAB_ROWS = BASE_P12 + PLANE_ROWS + 1    # +1 row slack for overlap reads


def build_bass():
    nc = bacc.Bacc("TRN2", target_bir_lowering=False, debug=False,
                   num_devices=NCORES)
    xin = nc.dram_tensor("xin", [128, JTOT * 4], F32, kind="ExternalInput")
    tab = nc.dram_tensor("tab", [TAB_ROWS, 128], F16, kind="ExternalInput")
    ltab = nc.dram_tensor("ltab", [L1, 2 * NF], F32, kind="ExternalInput")
    xl16 = nc.dram_tensor("xl16", [16, JTOT * 8], F32, kind="ExternalInput")
    out = nc.dram_tensor("out", [128, JTOT * NF], F32, kind="ExternalOutput")

    J = CHUNK_J
    with tile.TileContext(nc) as tc:
        import contextlib
        with contextlib.ExitStack() as ctx:
            xp = ctx.enter_context(tc.tile_pool(name="xp", bufs=2))
            wp = ctx.enter_context(tc.tile_pool(name="wp", bufs=2))
            sp = ctx.enter_context(tc.tile_pool(name="sp", bufs=2))
            op = ctx.enter_context(tc.tile_pool(name="op", bufs=3))
            gp = ctx.enter_context(tc.tile_pool(name="gp", bufs=4))
            mp = ctx.enter_context(tc.tile_pool(name="mp", bufs=2))
            rp = ctx.enter_context(tc.tile_pool(name="rp", bufs=2))
            lp = ctx.enter_context(tc.tile_pool(name="lp", bufs=4))
            lip = ctx.enter_context(tc.tile_pool(name="lip", bufs=2))

            for c in range(NCHUNK):
                j0 = c * J
                # ---- load x chunk: [128, J, 4]
                xs = xp.tile([128, J, 4], F32, tag="xs")
                nc.sync.dma_start(
                    xs[:], xin.ap()[:, j0 * 4:(j0 + J) * 4]
                           .rearrange("p (j c) -> p j c", c=4))

                def coord(k):
                    return xs[:, :, k:k + 1].rearrange("p j o -> p (j o)")

                # ---- per-coord floors and fracs
                def floorfrac(fv, tg):
                    ri = sp.tile([128, J], I32, tag="ffi", name="ri")
                    nc.vector.tensor_copy(ri[:], fv[:])          # round
                    rf = sp.tile([128, J], F32, tag="ffr", name="rf")
                    nc.vector.tensor_copy(rf[:], ri[:])
                    m = sp.tile([128, J], F32, tag="ffm", name="m")
                    nc.vector.tensor_tensor(out=m[:], in0=rf[:], in1=fv[:],
                                            op=ALU.is_gt)
                    fl = sp.tile([128, J], F32, tag=tg + "l", name="fl")
                    nc.vector.tensor_sub(fl[:], rf[:], m[:])
                    w = sp.tile([128, J], F32, tag=tg + "w", name="w")
                    nc.vector.tensor_sub(w[:], fv[:], fl[:])
                    return fl, w

                fl3, w3, fl2, w2 = [], [], [], []
                for k in range(3):
                    t = sp.tile([128, J], F32, tag="t")
                    nc.vector.tensor_scalar(out=t[:], in0=coord(k),
                                            scalar1=1.0, scalar2=0.5,
                                            op0=ALU.add, op1=ALU.mult)
                    f3 = sp.tile([128, J], F32, tag="f3")
                    nc.vector.tensor_scalar(out=f3[:], in0=t[:],
                                            scalar1=float(D3 - 1), scalar2=None,
                                            op0=ALU.mult)
                    f2 = sp.tile([128, J], F32, tag="f2")
                    nc.vector.tensor_scalar(out=f2[:], in0=t[:],
                                            scalar1=float(P2 - 1), scalar2=None,
                                            op0=ALU.mult)
                    a, b_ = floorfrac(f3, f"f3{k}")
                    fl3.append(a); w3.append(b_)
                    a, b_ = floorfrac(f2, f"f2{k}")
                    fl2.append(a); w2.append(b_)

                # ---- gather offsets (fp32 -> int32), in 256B-row units
                # grid row index = z*D3*D3 + y*D3 + x
                offg = op.tile([128, J], I32, tag="offg")
                b_ = sp.tile([128, J], F32, tag="gb")
                nc.vector.tensor_scalar(out=b_[:], in0=fl3[1],
                                        scalar1=float(D3), scalar2=None,
                                        op0=ALU.mult)
                a_ = sp.tile([128, J], F32, tag="ga")
                nc.vector.scalar_tensor_tensor(
                    out=a_[:], in0=fl3[2], scalar=float(D3 * D3), in1=b_[:],
                    op0=ALU.mult, op1=ALU.add)
                g00 = sp.tile([128, J], F32, tag="g00")
                nc.vector.tensor_add(g00[:], a_[:], fl3[0])
                nc.vector.tensor_copy(offg[:], g00[:])

                offp = op.tile([128, 3 * J], I32, tag="offp")
                for p_i, (ky, kx, base) in enumerate(
                        ((1, 0, BASE_P01), (2, 0, BASE_P02), (2, 1, BASE_P12))):
                    r_ = sp.tile([128, J], F32, tag="pr")
                    nc.vector.scalar_tensor_tensor(
                        out=r_[:], in0=fl2[ky], scalar=float(P2), in1=fl2[kx],
                        op0=ALU.mult, op1=ALU.add)
                    r2 = sp.tile([128, J], F32, tag="pr2")
                    nc.vector.tensor_scalar(out=r2[:], in0=r_[:],
                                            scalar1=1.0, scalar2=float(base),
                                            op0=ALU.mult, op1=ALU.add)
                    nc.vector.tensor_copy(offp[:, p_i * J:(p_i + 1) * J], r2[:])

                # ---- weight pairs (fp16) and corner products
                def mkpair(w, tag):
                    pr = wp.tile([128, J, 2], F16, tag=tag)
                    nc.vector.tensor_copy(pr[:, :, 1:2],
                                          w[:].to_broadcast([128, J, 1]))
                    nc.vector.tensor_scalar(
                        out=pr[:, :, 0:1],
                        in0=w[:].to_broadcast([128, J, 1]),
                        scalar1=1.0, scalar2=-1.0,
                        op0=ALU.subtract, op1=ALU.mult)
                    return pr

                wzp = mkpair(w3[2], "wzp")
                wyp = mkpair(w3[1], "wyp")
                wxp = mkpair(w3[0], "wxp")
                w2xp = mkpair(w2[0], "w2xp")
                w2yp = mkpair(w2[1], "w2yp")
                w2zp = mkpair(w2[2], "w2zp")

                def outer2(pa, pb, tag):
                    # out[p, j, a, b] = pa[p,j,a] * pb[p,j,b]
                    o = wp.tile([128, J, 2, 2], F16, tag=tag)
                    nc.vector.tensor_mul(
                        o[:], pa[:].to_broadcast([128, J, 2, 2]),
                        AP(pb[:].tensor, pb[:].offset,
                           [list(pb[:].ap[0]), list(pb[:].ap[1]),
                            [0, 2], list(pb[:].ap[2])]))
                    return o

                # grid corner order within 512B read: (dx, dz, dy)
                wzy = outer2(wzp, wyp, "wzy")          # [p,J,dz,dy]
                w8 = wp.tile([128, J, 2, 2, 2], F16, tag="w8")  # [dx,dz,dy]
                nc.vector.tensor_mul(
                    w8[:],
                    AP(wxp[:].tensor, wxp[:].offset,
                       [list(wxp[:].ap[0]), list(wxp[:].ap[1]),
                        list(wxp[:].ap[2]), [0, 2], [0, 2]]),
                    AP(wzy[:].tensor, wzy[:].offset,
                       [list(wzy[:].ap[0]), list(wzy[:].ap[1]), [0, 2],
                        list(wzy[:].ap[2]), list(wzy[:].ap[3])]))
                # plane brick corner order: (dy, dx) [row-pairs: y, y+1]
                # brick = [P(y,x), P(y,x+1), P(y+1,x), P(y+1,x+1)]
                w401 = outer2(w2yp, w2xp, "w401")
                w402 = outer2(w2zp, w2xp, "w402")
                w412 = outer2(w2zp, w2yp, "w412")

                # line weights stay f32
                flv = sp.tile([128, J], F32, tag="flv")
                nc.vector.tensor_scalar(out=flv[:], in0=coord(3),
                                        scalar1=float(L1), scalar2=None,
                                        op0=ALU.mult)
                _, wl = floorfrac(flv, "flx")
                wlp = wp.tile([128, J, 2], F32, tag="wlp")
                nc.vector.tensor_copy(wlp[:, :, 1:2],
                                      wl[:].to_broadcast([128, J, 1]))
                nc.vector.tensor_scalar(
                    out=wlp[:, :, 0:1],
                    in0=wl[:].to_broadcast([128, J, 1]),
                    scalar1=1.0, scalar2=-1.0,
                    op0=ALU.subtract, op1=ALU.mult)

                # ---- line: batched dma_gather (int16 idx on 16 partitions)
                li_f = lip.tile([16, J * 8], F32, tag="lif")
                nc.sync.dma_start(
                    li_f[:], xl16.ap()[:, c * J * 8:(c + 1) * J * 8])
                lfv = lip.tile([16, J * 8], F32, tag="lfv")
                nc.vector.tensor_scalar(out=lfv[:], in0=li_f[:],
                                        scalar1=float(L1), scalar2=None,
                                        op0=ALU.mult)
                lri = lip.tile([16, J * 8], I32, tag="lri")
                nc.vector.tensor_copy(lri[:], lfv[:])
                lrf = lip.tile([16, J * 8], F32, tag="lrf")
                nc.vector.tensor_copy(lrf[:], lri[:])
                lm = lip.tile([16, J * 8], F32, tag="lm")
                nc.vector.tensor_tensor(out=lm[:], in0=lrf[:], in1=lfv[:],
                                        op=ALU.is_gt)
                lfl = lip.tile([16, J * 8], F32, tag="lfl")
                nc.vector.tensor_sub(lfl[:], lrf[:], lm[:])
                lidx = lip.tile([128, J * 8], I16, tag="lidx")
                nc.vector.tensor_copy(lidx[0:16, :], lfl[:])
                for d in range(3):
                    n = 16 << d
                    nc.sync.dma_start(lidx[n:2 * n, :], lidx[0:n, :])
                ld = lp.tile([128, J, 2 * NF], F32, tag="ld")
                nc.gpsimd.dma_gather(
                    out_ap=ld[:], in_ap=ltab.ap(), idxs_ap=lidx[:],
                    num_idxs=J * 128, num_idxs_reg=J * 128,
                    elem_size=2 * NF, single_packet=False)

                # ---- per-j-column indirect gathers (fp16)
                g3c = gp.tile([128, J, 256], F16, tag="g3")
                gplc = [gp.tile([128, J, 128], F16, tag=f"gp{i}",
                                name=f"gp{i}") for i in range(3)]
                for j in range(J):
                    nc.gpsimd.indirect_dma_start(
                        out=g3c[:, j:j + 1, :].rearrange("p a b -> p (a b)"),
                        out_offset=None, in_=tab.ap(),
                        in_offset=IndirectOffsetOnAxis(
                            ap=offg[:, j:j + 1], axis=0))
                    for p_i in range(3):
                        nc.gpsimd.indirect_dma_start(
                            out=gplc[p_i][:, j:j + 1, :]
                                .rearrange("p a b -> p (a b)"),
                            out_offset=None, in_=tab.ap(),
                            in_offset=IndirectOffsetOnAxis(
                                ap=offp[:, p_i * J + j:p_i * J + j + 1],
                                axis=0))

                # ---- compute per tile (fp16 mult + tree adds)
                oc = rp.tile([128, J, NF], F32, tag="oc")
                for s in range(NTILE):
                    u0 = s * TILE_S
                    S = TILE_S

                    def wsum16(gt, wt, ncorn, tag):
                        # gt: [p,S,ncorn*NF] f16; wt [p,J,ncorn] f16
                        m = mp.tile([128, S, ncorn, NF], F16, tag=tag)
                        nc.vector.tensor_mul(
                            m[:],
                            gt.rearrange("p u (c f) -> p u c f",
                                         c=ncorn, f=NF),
                            AP(wt.tensor, wt.offset + u0 * ncorn,
                               [list(wt.ap[0]), [ncorn, S],
                                [1, ncorn], [0, NF]]))
                        # tree-add over corners
                        cur = m
                        n = ncorn
                        while n > 1:
                            h = mp.tile([128, S, n // 2, NF], F16,
                                        tag=tag + f"h{n}")
                            nc.vector.tensor_add(
                                h[:], cur[:, :, 0:n // 2, :],
                                cur[:, :, n // 2:n, :])
                            cur = h
                            n //= 2
                        return cur[:, :, 0:1, :].rearrange(
                            "p u o f -> p u (o f)")

                    f3r = wsum16(g3c[:, u0:u0 + S, :],
                                 w8[:].rearrange("p j a b c -> p j (a b c)"),
                                 8, "m8")
                    p01r = wsum16(gplc[0][:, u0:u0 + S, :],
                                  w401[:].rearrange("p j a b -> p j (a b)"),
                                  4, "m01")
                    p02r = wsum16(gplc[1][:, u0:u0 + S, :],
                                  w402[:].rearrange("p j a b -> p j (a b)"),
                                  4, "m02")
                    p12r = wsum16(gplc[2][:, u0:u0 + S, :],
                                  w412[:].rearrange("p j a b -> p j (a b)"),
                                  4, "m12")

                    # line weighted sum in f32
                    ml = mp.tile([128, S, 2, NF], F32, tag="ml")
                    nc.vector.tensor_mul(
                        ml[:],
                        ld[:, u0:u0 + S, :].rearrange(
                            "p u (c f) -> p u c f", c=2, f=NF),
                        AP(wlp[:].tensor, wlp[:].offset + u0 * 2,
                           [list(wlp[:].ap[0]), [2, S], [1, 2], [0, NF]]))
                    flr = mp.tile([128, S, NF], F32, tag="mlr")
                    nc.vector.tensor_add(flr[:], ml[:, :, 0:1, :].rearrange(
                        "p u o f -> p u (o f)"), ml[:, :, 1:2, :].rearrange(
                        "p u o f -> p u (o f)"))

                    o1 = rp.tile([128, S, NF], F16, tag="o1")
                    nc.vector.tensor_mul(o1[:], f3r, p01r)
                    o2 = rp.tile([128, S, NF], F16, tag="o2")
                    nc.vector.tensor_mul(o2[:], p02r, p12r)
                    o3 = rp.tile([128, S, NF], F32, tag="o3")
                    nc.vector.tensor_mul(o3[:], o1[:], o2[:])
                    nc.vector.tensor_mul(oc[:, u0:u0 + S, :], o3[:], flr[:])
                nc.sync.dma_start(
                    out.ap()[:, j0 * NF:(j0 + J) * NF],
                    oc[:].rearrange("p u f -> p (u f)"))

    nc.compile()
    return nc


def _prep_tables(grid3d, plane01, plane02, plane12, line0):
    # grid: zy-shingled fp16 rows: row(z,y,x) = [v(z,y,x), v(z,y+1,x),
    #   v(z+1,y,x), v(z+1,y+1,x)], each 32 fp16
    g = np.ascontiguousarray(grid3d.transpose(1, 2, 3, 0)).astype(np.float16)
    S = np.zeros((D3, D3, D3, 4, NF), np.float16)
    S[:, :, :, 0] = g
    S[:, :-1, :, 1] = g[:, 1:]
    S[:-1, :, :, 2] = g[1:]
    S[:-1, :-1, :, 3] = g[1:, 1:]
    grid_t = S.reshape(GRID_ROWS, 128)
    del S

    def shingle_plane(p):
        pt = np.ascontiguousarray(p.transpose(1, 2, 0)).astype(np.float16)
        ps = np.zeros((P2, P2, 4, NF), np.float16)
        ps[:, :, 0] = pt
        ps[:, :-1, 1] = pt[:, 1:]
        ps[:-1, :, 2] = pt[1:]
        ps[:-1, :-1, 3] = pt[1:, 1:]
        return ps.reshape(PLANE_ROWS, 128)

    tab = np.concatenate([grid_t, shingle_plane(plane01),
                          shingle_plane(plane02), shingle_plane(plane12),
                          np.zeros((1, 128), np.float16)], axis=0)
    lt = np.ascontiguousarray(line0.T)                        # (256, 32)
    ls = np.empty((L1, 2 * NF), np.float32)
    ls[:, :NF] = lt
    ls[:-1, NF:] = lt[1:]
    ls[-1, NF:] = lt[-1]
    return tab, ls


_NC_CACHE = {}


def kernel(x, grid3d, plane01, plane02, plane12, line0):
    x = np.asarray(x, np.float32)
    tab, ls = _prep_tables(np.asarray(grid3d, np.float32),
                           np.asarray(plane01, np.float32),
                           np.asarray(plane02, np.float32),
                           np.asarray(plane12, np.float32),
                           np.asarray(line0, np.float32))
    if "nc" not in _NC_CACHE:
        _NC_CACHE["nc"] = build_bass()
    nc = _NC_CACHE["nc"]

    in_maps = []
    for cix in range(NCORES):
        xc = x[cix * BCORE:(cix + 1) * BCORE]          # (131072, 4)
        xin = np.ascontiguousarray(xc.reshape(128, JTOT * 4))
        x3 = xc[:, 3].reshape(128, JTOT)               # [p, jglob]
        x3g = x3.reshape(8, 16, NCHUNK, CHUNK_J)       # [k, q, c, j]
        xl16 = np.ascontiguousarray(
            x3g.transpose(1, 2, 3, 0).reshape(16, JTOT * 8))
        in_maps.append({"xin": xin, "tab": tab, "ltab": ls, "xl16": xl16})

    res = run_bass_kernel_spmd(nc, in_maps, core_ids=list(range(NCORES)))
    outs = [r["out"].reshape(BCORE, NF) for r in res.results]
    return np.concatenate(outs, axis=0)
